# revision 59
# baseline (speedup 1.0000x reference)
import os
import sys

os.environ.setdefault("MYCRO_LOCAL_CACHE", "1")
sys.path.insert(0, "/opt/trn_rl_repo")

from contextlib import ExitStack

import numpy as np

import concourse.bass as bass
import concourse.mybir as mybir
import concourse.tile as tile
from concourse.bacc import Bacc
from concourse.bass_utils import run_bass_kernel_spmd

LAST_RESULT = None

F32 = mybir.dt.float32
F32R = mybir.dt.float32r
EXP = mybir.ActivationFunctionType.Exp
IDENT = mybir.ActivationFunctionType.Identity

B, N, C = 64, 197, 768
H, D = 12, 64
P = 10
M_KV = P + N            # 207
NCORES = 8
BC = B // NCORES        # 8 batches per core
T = BC * N              # 1576 tokens per core
GB = 2                  # batches per group (394 tokens -> f32r full rate)
NG = BC // GB           # 4 groups
GT = GB * N             # 394
TK0 = 128               # first kv token chunk
TK1 = N - TK0           # 69, second kv token chunk
PR0 = 96                # prefix row offset (legal SBUF start partition)
KVR = PR0 + P           # 106 rows: chunk2 at 0:69, zeros 69:96, prefix 96:106
KBZ0 = N                # 197: start of zero columns in kb
KBP0 = TK0 + PR0        # 224: start of prefix columns in kb
KB_W = KBP0 + P         # 234: kb = [tokens 0:197 | zeros | prefix]


def _build():
    # Bacc, not plain Bass: its compile() legalizes sync waits
    # (move_matmul_waits_to_ldweights / events) — walrus codegen allows
    # only one wait command per PE instruction
    nc = Bacc("TRN2", target_bir_lowering=False)
    xt = nc.dram_tensor("xt", [C, T], F32, kind="ExternalInput")
    wqk = nc.dram_tensor("wqk", [C, 2 * C], F32, kind="ExternalInput")
    wv = nc.dram_tensor("wv", [C, C], F32, kind="ExternalInput")
    wp = nc.dram_tensor("wp", [C, C], F32, kind="ExternalInput")
    pb = nc.dram_tensor("pb", [128, 6], F32, kind="ExternalInput")
    kpre = nc.dram_tensor("kpre", [128, BC * 6 * P], F32, kind="ExternalInput")
    vpre = nc.dram_tensor("vpre", [P, BC * C], F32, kind="ExternalInput")
    ot = nc.dram_tensor("ot", [C, T], F32, kind="ExternalOutput")

    with tile.TileContext(nc) as tc, ExitStack() as est:
        def pool(name, bufs, space=None):
            kw = {"space": space} if space is not None else {}
            return est.enter_context(tc.tile_pool(name=name, bufs=bufs, **kw))

        p_wqk = pool("p_wqk", 6)
        p_wv = pool("p_wv", 6)
        p_wp = pool("p_wp", 6)
        p_pb = pool("p_pb", 1)
        p_ones = pool("p_ones", 1)
        p_kpre = pool("p_kpre", 1)
        p_x = pool("p_x", 12)
        p_qk = pool("p_qk", 12)
        p_kb = pool("p_kb", 2)
        p_vkv = pool("p_vkv", 1)
        p_exps = pool("p_exps", 1)
        p_recip = pool("p_recip", 1)
        p_pbc = pool("p_pbc", 2)
        p_attn = pool("p_attn", 2)
        p_osb = pool("p_osb", 2)
        ps_mm = pool("ps_mm", 2, bass.MemorySpace.PSUM)
        ps_s = pool("ps_s", 2, bass.MemorySpace.PSUM)
        ps_o = pool("ps_o", 1, bass.MemorySpace.PSUM)
        ps_bd = pool("ps_bd", 1, bass.MemorySpace.PSUM)
        if True:
            wqk_t = []
            for k in range(6):
                t_ = p_wqk.tile([128, 2 * C], F32R)
                nc.sync.dma_start(t_[:], wqk[k * 128:(k + 1) * 128, :].bitcast(F32R))
                wqk_t.append(t_)
            wv_t = []
            for k in range(6):
                t_ = p_wv.tile([128, C], F32R)
                nc.sync.dma_start(t_[:], wv[k * 128:(k + 1) * 128, :].bitcast(F32R))
                wv_t.append(t_)
            wp_t = []
            for k in range(6):
                t_ = p_wp.tile([128, C], F32R)
                nc.sync.dma_start(t_[:], wp[k * 128:(k + 1) * 128, :].bitcast(F32R))
                wp_t.append(t_)
            pb_t = p_pb.tile([128, 6], F32)
            nc.sync.dma_start(pb_t[:], pb[:])
            ones = p_ones.tile([128, 64], F32R)
            nc.vector.memset(ones[:].bitcast(F32), 1.0)
            kpre_t = p_kpre.tile([128, BC * 6 * P], F32R)
            nc.sync.dma_start(kpre_t[:], kpre[:].bitcast(F32R))

            for g in range(NG):
                xg = []
                for k in range(6):
                    t_ = p_x.tile([128, GT], F32R)
                    nc.sync.dma_start(
                        t_[:], xt[k * 128:(k + 1) * 128, g * GT:(g + 1) * GT].bitcast(F32R)
                    )
                    xg.append(t_)

                # q,k projection: rows 0:768 = q (h*64+d), 768:1536 = k
                # q slabs stay [128, GT]; k is split per batch into kb tiles
                # [tokens 0:197 | zeros 197:224 | prefix 224:234]
                qk = []
                kb = {}
                for oc in range(12):
                    ps = ps_mm.tile([128, GT], F32)
                    for k in range(6):
                        nc.tensor.matmul(
                            ps[:],
                            wqk_t[k][:, oc * 128:(oc + 1) * 128],
                            xg[k][:],
                            start=(k == 0), stop=(k == 5),
                        )
                    if oc < 6:
                        sb = p_qk.tile([128, GT], F32R)
                        nc.vector.tensor_copy(sb[:], ps[:])
                        qk.append(sb)
                    else:
                        j = oc - 6
                        for bb in range(GB):
                            t_ = p_kb.tile([128, KB_W], F32R, name=f"kb{j}_{bb}")
                            nc.vector.memset(t_[:, KBZ0:KBP0].bitcast(F32), 0.0)
                            nc.vector.tensor_copy(
                                t_[:, 0:N], ps[:, bb * N:(bb + 1) * N]
                            )
                            idx = ((g * GB + bb) * 6 + j) * P
                            nc.vector.tensor_copy(
                                t_[:, KBP0:KB_W], kpre_t[:, idx:idx + P]
                            )
                            kb[(bb, j)] = t_

                attn_tiles = [p_attn.tile([128, GT], F32R, name=f"attn_{i}") for i in range(6)]

                for bb in range(GB):
                    b = g * GB + bb
                    col0 = bb * N

                    # v in natural [token, c] layout; vk0 = tokens 0:128,
                    # vkP = tokens 128:197 at rows 0:69, zeros, prefix at 96:106
                    vk0 = p_vkv.tile([128, C], F32R, name="vk0")
                    vkP = p_vkv.tile([KVR, C], F32R, name="vkP")
                    nc.vector.memset(vkP[64:PR0, :].bitcast(F32), 0.0)
                    nc.sync.dma_start(
                        vkP[PR0:PR0 + P, :], vpre[:, b * C:(b + 1) * C].bitcast(F32R)
                    )
                    for (ts_, tn, dst, ro) in ((0, TK0, vk0, 0), (TK0, TK1, vkP, 0)):
                        for nch in range(2):
                            ps = ps_mm.tile([tn, 384], F32, name="ps")
                            for k in range(6):
                                nc.tensor.matmul(
                                    ps[:],
                                    xg[k][:, col0 + ts_:col0 + ts_ + tn],
                                    wv_t[k][:, nch * 384:(nch + 1) * 384],
                                    start=(k == 0), stop=(k == 5),
                                )
                            nc.vector.tensor_copy(
                                dst[ro:ro + tn, nch * 384:(nch + 1) * 384], ps[:]
                            )

                    # scores, rhs widened to the full 394-token group slab
                    # (junk half ignored at the exp read)
                    exq = p_exps.tile([TK0, H * N], F32R, name="exq")
                    exPb = p_exps.tile([KVR, H * N], F32R, name="exPb")
                    nc.vector.memset(exPb[64:PR0, :].bitcast(F32), 0.0)
                    for h in range(H):
                        po = (h % 2) * 64
                        q_full = qk[h // 2][po:po + 64, :]
                        kbt = kb[(bb, h // 2)]
                        ps0 = ps_s.tile([TK0, 512], F32, name="ps0")
                        nc.tensor.matmul(
                            ps0[:, 0:GT], kbt[po:po + 64, 0:TK0],
                            q_full, start=True, stop=True,
                        )
                        nc.scalar.activation(
                            exq[:, h * N:(h + 1) * N], ps0[:, col0:col0 + N],
                            EXP, scale=0.125,
                        )
                        psC = ps_s.tile([KVR, 512], F32, name="psC", bufs=2)
                        nc.tensor.matmul(
                            psC[:, 0:GT], kbt[po:po + 64, TK0:KB_W],
                            q_full, start=True, stop=True,
                        )
                        nc.scalar.activation(
                            exPb[PR0:PR0 + P, h * N:(h + 1) * N],
                            psC[PR0:PR0 + P, col0:col0 + N], EXP, scale=0.125,
                        )
                        nc.scalar.activation(
                            exPb[0:TK1, h * N:(h + 1) * N],
                            psC[0:TK1, col0:col0 + N], EXP, scale=0.125,
                        )

                    # softmax denominators for all heads: ones-column matmuls
                    rc = p_recip.tile([1, H * N], F32R)
                    off = 0
                    while off < H * N:
                        nsz = min(512, H * N - off)
                        pd = ps_bd.tile([1, nsz], F32, name="pd")
                        nc.tensor.matmul(
                            pd[:], ones[0:TK0, 0:1], exq[:, off:off + nsz],
                            start=True, stop=False,
                        )
                        nc.tensor.matmul(
                            pd[:], ones[0:KVR, 0:1], exPb[:, off:off + nsz],
                            start=False, stop=True,
                        )
                        with nc.allow_low_precision(
                            reason="recip f32r feeds broadcast matmul"
                        ):
                            nc.vector.reciprocal(rc[:, off:off + nsz], pd[:])
                        off += nsz

                    # attn output (unnormalized), rhs widened to head pairs
                    pbcS = None
                    for h in range(H):
                        po = (h % 2) * 64
                        rs = (h // 2) * 2 * N
                        vo = (h % 2) * N
                        pso = ps_o.tile([64, 2 * N], F32, name="pso")
                        nc.tensor.matmul(
                            pso[:], vk0[:, h * 64:(h + 1) * 64],
                            exq[:, rs:rs + 2 * N], start=True, stop=False,
                        )
                        nc.tensor.matmul(
                            pso[:], vkP[:, h * 64:(h + 1) * 64],
                            exPb[:, rs:rs + 2 * N], start=False, stop=True,
                        )
                        if h % 2 == 0:
                            pbp = ps_bd.tile([64, 2 * N], F32, name="pbp", tag="pd")
                            nc.tensor.matmul(
                                pbp[:], ones[0:1, 0:64], rc[0:1, rs:rs + 2 * N],
                                start=True, stop=True,
                            )
                            pbcS = p_pbc.tile([64, 2 * N], F32, name="pbcS")
                            nc.vector.tensor_copy(pbcS[:], pbp[:])
                        nc.vector.tensor_mul(
                            attn_tiles[h // 2][po:po + 64, col0:col0 + N],
                            pso[:, vo:vo + N], pbcS[:, vo:vo + N],
                        )

                # output projection + bias
                for oc in range(6):
                    ps = ps_mm.tile([128, GT], F32, name="ps")
                    for k in range(6):
                        nc.tensor.matmul(
                            ps[:],
                            wp_t[k][:, oc * 128:(oc + 1) * 128],
                            attn_tiles[k][:],
                            start=(k == 0), stop=(k == 5),
                        )
                    ob = p_osb.tile([128, GT], F32)
                    nc.scalar.activation(ob[:], ps[:], IDENT, bias=pb_t[:, oc:oc + 1])
                    nc.sync.dma_start(ot[oc * 128:(oc + 1) * 128, g * GT:(g + 1) * GT], ob[:])

    return nc


def kernel(x, prompt, qkv_w, proj_w, proj_b):
    x = np.ascontiguousarray(x, dtype=np.float32)
    prompt = np.ascontiguousarray(prompt, dtype=np.float32)
    qkv_w = np.ascontiguousarray(qkv_w, dtype=np.float32)
    proj_w = np.ascontiguousarray(proj_w, dtype=np.float32)
    proj_b = np.ascontiguousarray(proj_b, dtype=np.float32)

    wqk_np = np.ascontiguousarray(qkv_w[:2 * C].T)
    wv_np = np.ascontiguousarray(qkv_w[2 * C:].T)
    wp_np = np.ascontiguousarray(proj_w.T)
    pb_np = np.ascontiguousarray(proj_b.reshape(6, 128).T)

    in_maps = []
    for c in range(NCORES):
        xs = x[c * BC:(c + 1) * BC]
        xt_np = np.ascontiguousarray(xs.transpose(2, 0, 1).reshape(C, T))
        pc = prompt[c * BC:(c + 1) * BC]
        # kpre rows: i*64+d for head 2j+i; cols: (b*6+j)*P+p
        a2 = np.ascontiguousarray(pc[:, 0]).reshape(BC, P, 6, 2, D)
        kpre_np = np.ascontiguousarray(
            a2.transpose(3, 4, 0, 2, 1).reshape(2 * D, BC * 6 * P)
        )
        vpre_np = np.ascontiguousarray(pc[:, 1].transpose(1, 0, 2, 3).reshape(P, BC * C))
        in_maps.append({
            "xt": xt_np, "wqk": wqk_np, "wv": wv_np, "wp": wp_np,
            "pb": pb_np, "kpre": kpre_np, "vpre": vpre_np,
        })

    nc = _build()
    if not nc.is_finalized():
        nc.finalize()
    try:
        res = run_bass_kernel_spmd(nc, in_maps, core_ids=list(range(NCORES)))
    except ModuleNotFoundError:
        # NTFF profile hook unavailable in this environment
        os.environ["BASS_NEVER_TRACE"] = "1"
        res = run_bass_kernel_spmd(nc, in_maps, core_ids=list(range(NCORES)))
    global LAST_RESULT
    LAST_RESULT = res
    outs = []
    for c in range(NCORES):
        ot_np = np.asarray(res.results[c]["ot"])
        outs.append(ot_np.reshape(C, BC, N).transpose(1, 2, 0))
    return np.ascontiguousarray(np.concatenate(outs, axis=0)).astype(np.float32)
